# revision 2
# baseline (speedup 1.0000x reference)
"""2-layer GAT (GATConv x2, 4 heads, concat) over a 100K-node / 1.7M-edge graph
on 8 Trainium2 NeuronCores.

Destination-sharded graph parallelism:
  - Nodes sharded 12500/core; core k owns destinations [12500k, 12500(k+1)).
  - Per layer each core computes h = x_in @ W.T for its own slice and an
    AllGather replicates the full feature table G [100352, 128] bf16 (256B
    rows) to every core.
  - Edge phase per core, per destination block (128 dsts): edges land in
    128-edge groups (partition-major) via dma_gather (int16 indices, the table
    is addressed in 4 chunks of 32768 rows). Attention pieces:
      s_e   = <h[src_e], att_src>   computed on DVE from the gathered rows
      d_e   = a_dst[dst_e]          broadcast by a PE matmul with a
                                    host-precomputed fp8 0/1 matrix S01dT[d,e]
      ex_e  = exp(leaky_relu(s_e + d_e))
    One PE matmul per group with an on-chip selection matrix S01T[e,d]
    accumulates numerators and softmax denominators together in PSUM:
      psum[d, 0:128] += sum_e S01T[e,d] * ex[e,h] * h[src_e][h,c]
      psum[d,128:132]+= sum_e S01T[e,d] * ex[e,h]
    Softmax normalization commutes with the linear aggregation and happens
    per destination after accumulation.
  - Self-loops are ordinary edges; weights replicated; the host does only
    data layout (sharding, sorting, padding, index tables, dtype casts).
"""

import numpy as np
import ml_dtypes

import concourse.bass as bass
import concourse.bacc as bacc
import concourse.tile as tile
import concourse.mybir as mybir
from concourse.bass import IndirectOffsetOnAxis
from concourse.bass_utils import run_bass_kernel_spmd

BF16 = ml_dtypes.bfloat16
FP8 = ml_dtypes.float8_e4m3

N = 100000
EMB_IN = 32
HEADS = 4
C = 32
HID = 128
NEG = 0.2
NCORES = 8
NSH = N // NCORES            # 12500 nodes per shard
NBLK = (NSH + 127) // 128    # 98 dst blocks per shard
NPAD = NBLK * 128            # 12544 padded rows per shard
CHUNK = 32768                # dma_gather int16 index range
NCHUNK = (NCORES * NPAD + CHUNK - 1) // CHUNK
GSUP = 2                     # blocks per super-batch
PAD_LOC = 300.0              # dstloc value for padding slots

_cache = {}

# debug knobs (bisecting bring-up): limit supers, run only layer 1
DBG_NSUP = None
DBG_L1_ONLY = False
DBG_STAGE = 4          # 1=gather only, 2=+bcast mm, 3=+ex, 4=full
DBG_LOCAL_TABLE = False  # copy AllGather output to a local table first


def _host_layout(x, edge_index):
    """Per-core edge/gather index tables. Pure index manipulation."""
    src = np.concatenate([edge_index[0], np.arange(N, dtype=np.int64)])
    dst = np.concatenate([edge_index[1], np.arange(N, dtype=np.int64)])
    trow = (src // NSH) * NPAD + (src % NSH)   # table row of source node

    percore = []
    cnt = np.zeros((NCORES, NBLK), dtype=np.int64)
    for k in range(NCORES):
        lo = k * NSH
        m = (dst >= lo) & (dst < lo + NSH)
        tr = trow[m]
        dl = dst[m] - lo
        blk = dl // 128
        order = np.argsort(blk, kind="stable")
        tr, dl, blk = tr[order], dl[order], blk[order]
        cnt[k] = np.bincount(blk, minlength=NBLK)
        percore.append((tr, dl))

    gb = np.maximum((np.max(cnt, axis=0) + 127) // 128, 1)   # groups per block
    gtot = int(gb.sum())
    blk_groups = []
    gg = 0
    for b in range(NBLK):
        blk_groups.append(list(range(gg, gg + int(gb[b]))))
        gg += int(gb[b])

    cores = []
    for k in range(NCORES):
        tr, dl = percore[k]
        bstart = np.concatenate([[0], np.cumsum(cnt[k])])
        rowm = np.zeros((gtot, 128), dtype=np.int32)
        locm = np.full((gtot, 128), PAD_LOC, dtype=np.float32)
        for b in range(NBLK):
            nb = int(cnt[k][b])
            G = int(gb[b])
            bs = np.zeros(G * 128, dtype=np.int32)
            bl = np.full(G * 128, PAD_LOC, dtype=np.float32)
            sl = slice(bstart[b], bstart[b] + nb)
            bs[:nb] = tr[sl]
            bl[:nb] = (dl[sl] - b * 128).astype(np.float32)
            g0 = blk_groups[b][0]
            rowm[g0:g0 + G] = bs.reshape(G, 128)
            locm[g0:g0 + G] = bl.reshape(G, 128)
        loci = locm.astype(np.int64)
        valid = loci < 128
        sdt = np.zeros((128, gtot * 128), dtype=FP8)
        ggi, ei = np.nonzero(valid)
        sdt[loci[valid], ggi * 128 + ei] = FP8(1.0)
        xsl = np.zeros(NPAD, dtype=np.int32)
        xsl[:NSH] = x[k * NSH:(k + 1) * NSH].astype(np.int32)
        cores.append(dict(gsrc=np.ascontiguousarray(rowm.T).astype(np.int32),
                          dstloc=np.ascontiguousarray(locm.T).astype(BF16),
                          sdt=np.ascontiguousarray(sdt),
                          xsh=xsl.reshape(NBLK, 128).T.copy()))
    meta = dict(blk_groups=blk_groups, gtot=gtot, gb=gb)
    return meta, cores


def _build(nc, meta):
    dt = mybir.dt
    f32, bf16, i32, i16 = dt.float32, dt.bfloat16, dt.int32, dt.int16
    f8 = dt.float8e4
    gtot = meta["gtot"]
    blk_groups = meta["blk_groups"]

    emb_t = nc.dram_tensor("emb", [N, EMB_IN], f32, kind="ExternalInput")
    w1_t = nc.dram_tensor("w1", [HID, EMB_IN], f32, kind="ExternalInput")
    w1t_t = nc.dram_tensor("w1t", [EMB_IN, HID], f32, kind="ExternalInput")
    w2_t = nc.dram_tensor("w2", [HID, HID], f32, kind="ExternalInput")
    w2t_t = nc.dram_tensor("w2t", [HID, HID], f32, kind="ExternalInput")
    a1d_t = nc.dram_tensor("a1d", [HID, HEADS], f32, kind="ExternalInput")
    a2d_t = nc.dram_tensor("a2d", [HID, HEADS], f32, kind="ExternalInput")
    at1_t = nc.dram_tensor("attrep1", [128, HID], bf16, kind="ExternalInput")
    at2_t = nc.dram_tensor("attrep2", [128, HID], bf16, kind="ExternalInput")
    b1r_t = nc.dram_tensor("b1r", [128, HID], f32, kind="ExternalInput")
    b2r_t = nc.dram_tensor("b2r", [128, HID], f32, kind="ExternalInput")
    iota_t = nc.dram_tensor("iota128", [128, 128], bf16, kind="ExternalInput")
    idf_t = nc.dram_tensor("identf", [128, 128], f32, kind="ExternalInput")
    xsh_t = nc.dram_tensor("xsh", [128, NBLK], i32, kind="ExternalInput")
    gsrc_t = nc.dram_tensor("gsrc", [128, gtot], i32, kind="ExternalInput")
    sdt_t = nc.dram_tensor("sdt", [128, gtot * 128], f8, kind="ExternalInput")
    dstloc_t = nc.dram_tensor("dstloc", [128, gtot], bf16, kind="ExternalInput")
    out_t = nc.dram_tensor("out2", [NPAD, HID], f32, kind="ExternalOutput")

    gslice = [nc.dram_tensor(f"gslice{l}", [NPAD, HID], bf16, kind="Internal")
              for l in (1, 2)]
    gfull = [nc.dram_tensor(f"gfull{l}", [NCORES * NPAD, HID], bf16,
                            kind="Internal", addr_space="Shared") for l in (1, 2)]

    def bmid(ap, w):
        """[128, X] -> [128, w, X] broadcast over a middle dim."""
        return bass.AP(ap.tensor, ap.offset, [list(ap.ap[0]), [0, w],
                                              list(ap.ap[1])])

    with tile.TileContext(nc) as tc:
        with tc.tile_pool(name="const", bufs=1) as cpool, \
             tc.tile_pool(name="work", bufs=2) as wpool, \
             tc.tile_pool(name="psum", bufs=2, space="PSUM") as ppool:

            def cload(t, shape, dtyp):
                s = cpool.tile(shape, dtyp, tag=t.name)
                nc.sync.dma_start(s[:], t[:])
                return s

            w1_sb = cload(w1_t, [HID, EMB_IN], f32)
            w1t_sb = cload(w1t_t, [EMB_IN, HID], f32)
            w2_sb = cload(w2_t, [HID, HID], f32)
            a1d_sb = cload(a1d_t, [HID, HEADS], f32)
            a2d_sb = cload(a2d_t, [HID, HEADS], f32)
            at1_sb = cload(at1_t, [128, HID], bf16)
            at2_sb = cload(at2_t, [128, HID], bf16)
            at_sb = [at1_sb, at2_sb]
            b1r_sb = cload(b1r_t, [128, HID], f32)
            b2r_sb = cload(b2r_t, [128, HID], f32)
            iota_sb = cload(iota_t, [128, 128], bf16)
            idf_sb = cload(idf_t, [128, 128], f32)
            xsh_sb = cload(xsh_t, [128, NBLK], i32)
            gsrc_sb = cload(gsrc_t, [128, gtot], i32)
            dstloc_sb = cload(dstloc_t, [128, gtot], bf16)

            w2t_f = cpool.tile([HID, HID], f32, tag="w2tf")
            nc.sync.dma_start(w2t_f[:], w2t_t[:])
            w2t_bf = cpool.tile([HID, HID], bf16, tag="w2tbf")
            nc.vector.tensor_copy(w2t_bf[:], w2t_f[:])

            # M1d = W1^T A1dst [32, 4];  M2d = W2^T A2dst [128, 4]
            m1_ps = ppool.tile([EMB_IN, HEADS], f32, tag="tp")
            nc.tensor.matmul(out=m1_ps[:], lhsT=w1_sb[:], rhs=a1d_sb[:],
                             start=True, stop=True)
            m1_sb = cpool.tile([EMB_IN, HEADS], f32, tag="m1s")
            nc.vector.tensor_copy(m1_sb[:], m1_ps[:])
            m2_ps = ppool.tile([HID, HEADS], f32, tag="tp")
            nc.tensor.matmul(out=m2_ps[:], lhsT=w2_sb[:], rhs=a2d_sb[:],
                             start=True, stop=True)
            m2_bf = cpool.tile([HID, HEADS], bf16, tag="m2b")
            nc.vector.tensor_copy(m2_bf[:], m2_ps[:])

            # a_dst for own blocks, per layer: [128, NBLK, 4]
            adsb1 = cpool.tile([128, NBLK, HEADS], bf16, tag="adsb1")
            adsb2 = cpool.tile([128, NBLK, HEADS], bf16, tag="adsb2")
            adsb = [adsb1, adsb2]

            # ---- phase A: layer-1 h slice for own nodes ------------------
            for g in range(NBLK):
                embx = wpool.tile([128, EMB_IN], f32, tag="embx")
                nc.gpsimd.indirect_dma_start(
                    out=embx[:], out_offset=None, in_=emb_t[:],
                    in_offset=IndirectOffsetOnAxis(ap=xsh_sb[:, g:g + 1], axis=0))
                tp = ppool.tile([EMB_IN, 128], f32, tag="tp")
                nc.tensor.transpose(tp[:], embx[:], idf_sb[:])
                exT = wpool.tile([EMB_IN, 128], f32, tag="exT")
                nc.vector.tensor_copy(exT[:], tp[:])
                hp = ppool.tile([128, HID + HEADS], f32, tag="hp")
                nc.tensor.matmul(out=hp[:, 0:HID], lhsT=exT[:], rhs=w1t_sb[:],
                                 start=True, stop=True)
                nc.tensor.matmul(out=hp[:, HID:HID + HEADS], lhsT=exT[:],
                                 rhs=m1_sb[:], start=True, stop=True)
                sl = wpool.tile([128, HID], bf16, tag="slice")
                nc.vector.tensor_copy(sl[:], hp[:, 0:HID])
                nc.vector.tensor_copy(adsb[0][:, g, :], hp[:, HID:HID + HEADS])
                nc.sync.dma_start(gslice[0][g * 128:(g + 1) * 128, :], sl[:])

            nc.gpsimd.collective_compute(
                "AllGather", mybir.AluOpType.bypass,
                replica_groups=[list(range(NCORES))],
                ins=[gslice[0].ap()], outs=[gfull[0].ap()])

            # ---- edge phase ---------------------------------------------
            tabrows = NCORES * NPAD
            layers = (0,) if DBG_L1_ONLY else (0, 1)
            gloc = [None, None]
            if DBG_LOCAL_TABLE:
                gloc[0] = nc.dram_tensor("gloc1", [NCORES * NPAD, HID], bf16,
                                         kind="Internal")
                if not DBG_L1_ONLY:
                    gloc[1] = nc.dram_tensor("gloc2", [NCORES * NPAD, HID],
                                             bf16, kind="Internal")
            for layer in layers:
                gf = gfull[layer]
                if DBG_LOCAL_TABLE:
                    nc.sync.dma_start(gloc[layer][:], gf[:])
                    gf = gloc[layer]
                for s in range(DBG_NSUP or (NBLK // GSUP)):
                    blks = list(range(s * GSUP, (s + 1) * GSUP))
                    plan_groups = [(0, b) for b in blks
                                   for _ in blk_groups[b]]
                    Ws = len(plan_groups)
                    gg0 = blk_groups[blks[0]][0]
                    gath = wpool.tile([128, Ws, HID], bf16, tag="gath")
                    for j in range(Ws):
                        nc.gpsimd.indirect_dma_start(
                            out=gath[:, j, :], out_offset=None, in_=gf[:],
                            in_offset=IndirectOffsetOnAxis(
                                ap=gsrc_sb[:, gg0 + j:gg0 + j + 1], axis=0))
                    sdt_sb = wpool.tile([128, Ws, 128], f8, tag="sdt")
                    nc.sync.dma_start(
                        sdt_sb[:],
                        sdt_t[:, gg0 * 128:(gg0 + Ws) * 128]
                        .rearrange("p (w e) -> p w e", w=Ws))
                    # d_e = a_dst[dst_e] via fp8 selection matmul
                    dps = ppool.tile([128, Ws, HEADS], f32, tag="dp")
                    for j, (c, b) in enumerate(plan_groups):
                        nc.tensor.matmul(out=dps[:, j, :], lhsT=sdt_sb[:, j, :],
                                         rhs=adsb[layer][:, b, :],
                                         start=True, stop=True)
                    if DBG_STAGE <= 2:
                        dmp = wpool.tile([128, HID], f32, tag="dmp")
                        nc.gpsimd.memset(dmp[:], 0.0)
                        nc.vector.tensor_copy(
                            dmp[:, 0:HEADS * min(Ws, 16)],
                            dps[:, 0:min(Ws, 16), :])
                        nc.sync.dma_start(
                            out_t[s * 128:(s + 1) * 128, :], dmp[:])
                        continue
                    # s_e = <h_src, att_src>
                    hm = wpool.tile([128, Ws, HID], bf16, tag="hm")
                    nc.vector.tensor_mul(hm[:], gath[:],
                                         bmid(at_sb[layer][:], Ws))
                    s_sb = wpool.tile([128, Ws, HEADS], f32, tag="s")
                    nc.vector.tensor_reduce(
                        s_sb[:], hm[:].rearrange("p w (h c) -> p w h c", h=HEADS),
                        axis=mybir.AxisListType.X, op=mybir.AluOpType.add)
                    # ex = exp(leaky_relu(s + d))
                    z = wpool.tile([128, Ws, HEADS], f32, tag="z")
                    nc.vector.tensor_add(z[:], s_sb[:], dps[:])
                    zm = wpool.tile([128, Ws, HEADS], f32, tag="zm")
                    nc.vector.tensor_scalar_mul(zm[:], z[:], NEG)
                    nc.vector.tensor_max(z[:], z[:], zm[:])
                    ex = wpool.tile([128, Ws, HEADS], bf16, tag="ex")
                    nc.scalar.activation(ex[:], z[:],
                                         mybir.ActivationFunctionType.Exp)
                    if DBG_STAGE <= 3:
                        dmp = wpool.tile([128, HID], f32, tag="dmp")
                        nc.gpsimd.memset(dmp[:], 0.0)
                        nc.vector.tensor_copy(
                            dmp[:, 0:HEADS * min(Ws, 16)],
                            ex[:, 0:min(Ws, 16), :])
                        nc.sync.dma_start(
                            out_t[s * 128:(s + 1) * 128, :], dmp[:])
                        continue
                    # rhs = [h * ex | ex]
                    rhs = wpool.tile([128, Ws, HID + HEADS], bf16, tag="rhs")
                    nc.vector.tensor_mul(
                        rhs[:, :, 0:HID].rearrange("p w (h c) -> p w h c", h=HEADS),
                        gath[:].rearrange("p w (h c) -> p w h c", h=HEADS),
                        ex[:].to_broadcast([128, Ws, HEADS, C]))
                    nc.vector.tensor_copy(rhs[:, :, HID:HID + HEADS], ex[:])
                    # selection matrices for aggregation
                    s01 = wpool.tile([128, Ws, 128], bf16, tag="s01")
                    nc.vector.tensor_tensor(
                        out=s01[:],
                        in0=dstloc_sb[:, gg0:gg0 + Ws].to_broadcast([128, Ws, 128]),
                        in1=bmid(iota_sb[:], Ws), op=mybir.AluOpType.is_equal)
                    # aggregate per block
                    for b in range(s * GSUP, (s + 1) * GSUP):
                        ggs = blk_groups[b]
                        agg = ppool.tile([128, HID + HEADS], f32, tag="agg")
                        for i, gg in enumerate(ggs):
                            j = gg - gg0
                            nc.tensor.matmul(out=agg[:], lhsT=s01[:, j, :],
                                             rhs=rhs[:, j, :],
                                             start=(i == 0),
                                             stop=(i == len(ggs) - 1))
                        den = wpool.tile([128, HEADS], f32, tag="den")
                        nc.vector.tensor_scalar_add(den[:],
                                                    agg[:, HID:HID + HEADS], 1e-16)
                        rec = wpool.tile([128, HEADS], f32, tag="rec")
                        nc.vector.reciprocal(rec[:], den[:])
                        normed = wpool.tile([128, HID], f32, tag="normed")
                        for h in range(HEADS):
                            nc.vector.tensor_mul(
                                normed[:, h * C:(h + 1) * C],
                                agg[:, h * C:(h + 1) * C],
                                rec[:, h:h + 1].to_broadcast([128, C]))
                        if layer == 0:
                            relu = wpool.tile([128, HID], f32, tag="relu")
                            nc.vector.tensor_add(normed[:], normed[:], b1r_sb[:])
                            nc.vector.tensor_scalar_max(relu[:], normed[:], 0.0)
                            if DBG_L1_ONLY:
                                nc.sync.dma_start(
                                    out_t[b * 128:(b + 1) * 128, :], relu[:])
                                continue
                            tp2 = ppool.tile([128, 128], f32, tag="tp")
                            nc.tensor.transpose(tp2[:], relu[:], idf_sb[:])
                            rT = wpool.tile([128, 128], bf16, tag="rT")
                            nc.vector.tensor_copy(rT[:], tp2[:])
                            hp2 = ppool.tile([128, HID + HEADS], f32, tag="hp")
                            nc.tensor.matmul(out=hp2[:, 0:HID], lhsT=rT[:],
                                             rhs=w2t_bf[:], start=True, stop=True)
                            nc.tensor.matmul(out=hp2[:, HID:HID + HEADS],
                                             lhsT=rT[:], rhs=m2_bf[:],
                                             start=True, stop=True)
                            sl2 = wpool.tile([128, HID], bf16, tag="slice")
                            nc.vector.tensor_copy(sl2[:], hp2[:, 0:HID])
                            nc.vector.tensor_copy(adsb[1][:, b, :],
                                                  hp2[:, HID:HID + HEADS])
                            nc.sync.dma_start(
                                gslice[1][b * 128:(b + 1) * 128, :], sl2[:])
                        else:
                            outb = wpool.tile([128, HID], f32, tag="outb")
                            nc.vector.tensor_add(outb[:], normed[:], b2r_sb[:])
                            nc.sync.dma_start(
                                out_t[b * 128:(b + 1) * 128, :], outb[:])
                if layer == 0 and not DBG_L1_ONLY:
                    nc.gpsimd.collective_compute(
                        "AllGather", mybir.AluOpType.bypass,
                        replica_groups=[list(range(NCORES))],
                        ins=[gslice[1].ap()], outs=[gfull[1].ap()])
    nc.finalize()
    return nc


def kernel(**inputs):
    x = np.asarray(inputs["x"])
    edge_index = np.asarray(inputs["edge_index"])
    emb = np.asarray(inputs["emb"], dtype=np.float32)
    W1 = np.asarray(inputs["W1"], dtype=np.float32)
    W2 = np.asarray(inputs["W2"], dtype=np.float32)
    as1 = np.asarray(inputs["att_src1"], dtype=np.float32)
    ad1 = np.asarray(inputs["att_dst1"], dtype=np.float32)
    as2 = np.asarray(inputs["att_src2"], dtype=np.float32)
    ad2 = np.asarray(inputs["att_dst2"], dtype=np.float32)
    b1 = np.asarray(inputs["b1"], dtype=np.float32)
    b2 = np.asarray(inputs["b2"], dtype=np.float32)

    key = (edge_index.tobytes(), x.tobytes())
    if _cache.get("key") != key:
        meta, cores = _host_layout(x, edge_index)
        nc = _build(bacc.Bacc("TRN2", target_bir_lowering=False, debug=False,
                              enable_asserts=False, num_devices=NCORES), meta)
        _cache.update(key=key, nc=nc, cores=cores)
    nc, cores = _cache["nc"], _cache["cores"]

    iota = np.broadcast_to(np.arange(128, dtype=np.float32), (128, 128))
    common = dict(
        emb=emb, w1=W1, w1t=np.ascontiguousarray(W1.T),
        w2=W2, w2t=np.ascontiguousarray(W2.T),
        a1d=_amat_d(ad1),
        a2d=_amat_d(ad2),
        attrep1=np.ascontiguousarray(
            np.broadcast_to(as1.reshape(-1), (128, HID))).astype(BF16),
        attrep2=np.ascontiguousarray(
            np.broadcast_to(as2.reshape(-1), (128, HID))).astype(BF16),
        b1r=np.ascontiguousarray(np.broadcast_to(b1, (128, HID))),
        b2r=np.ascontiguousarray(np.broadcast_to(b2, (128, HID))),
        iota128=np.ascontiguousarray(iota).astype(BF16),
        identf=np.eye(128, dtype=np.float32),
    )
    in_maps = [dict(common, **cores[k]) for k in range(NCORES)]

    res = run_bass_kernel_spmd(nc, in_maps, core_ids=list(range(NCORES)))
    global LAST_EXEC_NS, LAST_RESULT
    if getattr(res, "exec_time_ns", None) is not None:
        LAST_EXEC_NS = res.exec_time_ns
        LAST_RESULT = res
    out = np.concatenate([res.results[k]["out2"][:NSH] for k in range(NCORES)],
                         axis=0)
    return out.astype(np.float32)


def _amat_d(adst):
    A = np.zeros((HID, HEADS), dtype=np.float32)
    for h in range(HEADS):
        A[h * C:(h + 1) * C, h] = adst[h]
    return A


if __name__ == "__main__":
    import reference
    inputs = {k: np.asarray(v) for k, v in reference.setup_inputs().items()}
    got = kernel(**inputs)
    print("out shape", got.shape, got.dtype)



# revision 9
# speedup vs baseline: 1.0690x; 1.0690x over previous
"""2-layer GAT (GATConv x2, 4 heads, concat) over a 100K-node / 1.7M-edge graph
on 8 Trainium2 NeuronCores.

Destination-sharded graph parallelism, v2 (dma_gather edition):
  - Nodes sharded 12500/core; core k owns destinations [12500k, 12500(k+1)).
  - Layer-1 feature table is T = emb @ W1.T computed from SEQUENTIAL emb rows
    (host supplies a transposed, per-core zero-padded embT); the host remaps
    layer-1 edge gather indices to tablerow(x[src]) so no embedding x-gather
    is ever done on device.  Layer-2 table rows are indexed by src directly.
  - Per layer an AllGather replicates the full bf16 table G [100352, 128].
  - Edge phase per core, per super of SUP dst blocks: all edge rows land via
    <=4 chunked dma_gather calls (int16 idx local to a 25088-row chunk;
    128-edge groups bucketed host-side by (dst block, chunk); the group
    structure is shared by all cores = max over cores, per-core padded).
    Attention pieces:
      s_e  = <h[src_e], att_src>        DVE mul + reduce on gathered rows
      d_e  = a_dst[dst_e]               PE matmul with host fp8 sdt[d,e]
      ex_e = exp(leaky_relu(s_e+d_e))   ACT Lrelu + Exp into the rhs tail
    One PE matmul per group with host fp8 s01t[e,d] as lhsT accumulates
    numerators and softmax denominators together in PSUM:
      psum[d, 0:128] += sum_e s01t[e,d] * ex[e,h] * h[src_e][h,c]
      psum[d,128:132]+= sum_e s01t[e,d] * ex[e,h]
    Softmax normalization commutes with the linear aggregation and happens
    per destination after accumulation.
  - Self-loops are ordinary edges; weights replicated; the host does only
    data layout (sharding, sorting, padding, index tables, dtype casts).
"""

import numpy as np
import ml_dtypes

import concourse.bass as bass
import concourse.bacc as bacc
import concourse.tile as tile
import concourse.mybir as mybir
from concourse.bass import IndirectOffsetOnAxis
from concourse.bass_utils import run_bass_kernel_spmd

BF16 = ml_dtypes.bfloat16
FP8 = ml_dtypes.float8_e4m3

N = 100000
EMB_IN = 32
HEADS = 4
C = 32
HID = 128
NEG = 0.2
NCORES = 8
NSH = N // NCORES            # 12500 nodes per shard
NBLK = (NSH + 127) // 128    # 98 dst blocks per shard
NPAD = NBLK * 128            # 12544 padded rows per shard
TROWS = NCORES * NPAD        # 100352 table rows
NCHUNK = 4
CHUNK = TROWS // NCHUNK      # 25088 rows per dma_gather chunk (< 32768)
SUP = 4                      # dst blocks per super-batch

_cache = {}


def _trow(n):
    return (n // NSH) * NPAD + (n % NSH)


def _wrap16(flat):
    """flat [S*16] int -> [128, S] int16: stream j -> (partition j%16,
    col j//16), replicated across the 8 Q7 16-partition stripes."""
    S = len(flat) // 16
    w = flat.reshape(S, 16).astype(np.int16).T       # [16, S]
    return np.ascontiguousarray(np.tile(w, (8, 1)))  # [128, S]


def _supers():
    return [list(range(b, min(b + SUP, NBLK))) for b in range(0, NBLK, SUP)]


def _bucket_edges(trow_e, dl, order_blk):
    """Per (block, chunk): (chunk-local rows int array, dst-in-block locs)."""
    out = {}
    for b in range(NBLK):
        s, e = order_blk[b], order_blk[b + 1]
        tr = trow_e[s:e]
        dloc = dl[s:e] - b * 128
        cc = tr // CHUNK
        for c in range(NCHUNK):
            m = cc == c
            out[(b, c)] = (tr[m] % CHUNK, dloc[m])
    return out


def _common_structure(buckets_all):
    """G[(b,c)] = max over cores of ceil(n/128); group order and plan."""
    G = {}
    for b in range(NBLK):
        for c in range(NCHUNK):
            n = max(len(bk[(b, c)][0]) for bk in buckets_all)
            G[(b, c)] = (n + 127) // 128
    plan = []
    order = []          # flat list of (b, c) per group, in global col order
    gcol = 0
    for blks in _supers():
        cw = [0, 0, 0, 0]
        bg = {b: [] for b in blks}
        for c in range(NCHUNK):
            for b in blks:
                for _ in range(G[(b, c)]):
                    order.append((b, c))
                    bg[b].append(gcol)
                    gcol += 1
                    cw[c] += 1
        plan.append((cw, bg))
    return G, plan, order, gcol


def _core_tables(buckets, G, order, gtot):
    """Build idx16 / sdt / s01t for one core given the common structure."""
    idx = np.zeros((gtot, 128), dtype=np.int16)
    loc = np.full((gtot, 128), -1, dtype=np.int64)
    pos = {}
    for g, (b, c) in enumerate(order):
        pos.setdefault((b, c), []).append(g)
    for (b, c), gs in pos.items():
        tr, dloc = buckets[(b, c)]
        n = len(tr)
        cap = len(gs) * 128
        fi = np.zeros(cap, dtype=np.int16)
        fl = np.full(cap, -1, dtype=np.int64)
        fi[:n] = tr.astype(np.int16)
        fl[:n] = dloc
        for i, g in enumerate(gs):
            idx[g] = fi[i * 128:(i + 1) * 128]
            loc[g] = fl[i * 128:(i + 1) * 128]
    valid = loc >= 0
    gg, ei = np.nonzero(valid)
    dv = loc[valid]
    sdt = np.zeros((128, gtot * 128), dtype=FP8)     # [d, (g,e)]
    sdt[dv, gg * 128 + ei] = FP8(1.0)
    s01t = np.zeros((128, gtot * 128), dtype=FP8)    # [e, (g,d)]
    s01t[ei, gg * 128 + dv] = FP8(1.0)
    return _wrap16(idx.reshape(-1)), sdt, s01t


def _host_layout(x, edge_index):
    src = np.concatenate([edge_index[0], np.arange(N, dtype=np.int64)])
    dst = np.concatenate([edge_index[1], np.arange(N, dtype=np.int64)])
    trows = [_trow(x[src]), _trow(src)]   # layer-1: T[x[src]]; layer-2: src

    buckets = [[], []]   # [layer][core]
    xshs = []
    for k in range(NCORES):
        lo = k * NSH
        m = (dst >= lo) & (dst < lo + NSH)
        dl = dst[m] - lo
        blk = dl // 128
        order = np.argsort(blk, kind="stable")
        dl = dl[order]
        cnt = np.bincount(blk, minlength=NBLK)
        order_blk = np.concatenate([[0], np.cumsum(cnt)])
        for l in (0, 1):
            buckets[l].append(_bucket_edges(trows[l][m][order], dl, order_blk))
        xsh = np.zeros(NPAD, dtype=np.int32)
        xsh[:NSH] = _trow(x[lo:lo + NSH]).astype(np.int32)
        xshs.append(xsh.reshape(NBLK, 128).T.copy())

    meta = {"lay": []}
    percore = [dict(xsh=xshs[k]) for k in range(NCORES)]
    for l in (0, 1):
        G, plan, order, gtot = _common_structure(buckets[l])
        meta["lay"].append(dict(plan=plan, gtot=gtot))
        for k in range(NCORES):
            idx16, sdt, s01t = _core_tables(buckets[l][k], G, order, gtot)
            percore[k][f"gidx{l}"] = idx16
            percore[k][f"sdt{l}"] = sdt
            percore[k][f"s01{l}"] = s01t
    return meta, percore


def _bmid(ap, w):
    """[128, X] -> [128, w, X] broadcast over a middle dim."""
    return bass.AP(ap.tensor, ap.offset, [list(ap.ap[0]), [0, w],
                                          list(ap.ap[1])])


def _build(nc, meta):
    dt = mybir.dt
    f32, bf16, i32, i16 = dt.float32, dt.bfloat16, dt.int32, dt.int16
    f8 = dt.float8e4
    lay = meta["lay"]
    sups = _supers()

    embt_t = nc.dram_tensor("embt", [EMB_IN, NPAD], bf16, kind="ExternalInput")
    w1t_t = nc.dram_tensor("w1t", [EMB_IN, HID], bf16, kind="ExternalInput")
    w2t_t = nc.dram_tensor("w2t", [HID, HID], bf16, kind="ExternalInput")
    m2_t = nc.dram_tensor("m2", [HID, HEADS], bf16, kind="ExternalInput")
    ats1_t = nc.dram_tensor("ats1", [128, HID], bf16, kind="ExternalInput")
    ats2_t = nc.dram_tensor("ats2", [128, HID], bf16, kind="ExternalInput")
    atd1_t = nc.dram_tensor("atd1", [128, HID], bf16, kind="ExternalInput")
    b1r_t = nc.dram_tensor("b1r", [128, HID], f32, kind="ExternalInput")
    b2r_t = nc.dram_tensor("b2r", [128, HID], f32, kind="ExternalInput")
    idf_t = nc.dram_tensor("identf", [128, 128], f32, kind="ExternalInput")
    xsh_t = nc.dram_tensor("xsh", [128, NBLK], i32, kind="ExternalInput")
    gidx_t = [nc.dram_tensor(f"gidx{l}", [128, lay[l]["gtot"] * 8], i16,
                             kind="ExternalInput") for l in (0, 1)]
    sdt_t = [nc.dram_tensor(f"sdt{l}", [128, lay[l]["gtot"] * 128], f8,
                            kind="ExternalInput") for l in (0, 1)]
    s01_t = [nc.dram_tensor(f"s01{l}", [128, lay[l]["gtot"] * 128], f8,
                            kind="ExternalInput") for l in (0, 1)]
    out_t = nc.dram_tensor("out2", [NPAD, HID], f32, kind="ExternalOutput")

    gslice = [nc.dram_tensor(f"gslice{l}", [NPAD, HID], bf16, kind="Internal")
              for l in (1, 2)]
    gfull = [nc.dram_tensor(f"gfull{l}", [TROWS, HID], bf16,
                            kind="Internal", addr_space="Shared") for l in (1, 2)]

    with tile.TileContext(nc) as tc:
        with tc.tile_pool(name="const", bufs=1) as cpool, \
             tc.tile_pool(name="work", bufs=2) as wpool, \
             tc.tile_pool(name="psum", bufs=2, space="PSUM") as ppool:

            def cload(t, shape, dtyp):
                s = cpool.tile(shape, dtyp, tag=t.name)
                nc.sync.dma_start(s[:], t[:])
                return s

            w1t_sb = cload(w1t_t, [EMB_IN, HID], bf16)
            w2t_sb = cload(w2t_t, [HID, HID], bf16)
            m2_sb = cload(m2_t, [HID, HEADS], bf16)
            ats1_sb = cload(ats1_t, [128, HID], bf16)
            ats2_sb = cload(ats2_t, [128, HID], bf16)
            at_sb = [ats1_sb, ats2_sb]
            atd1_sb = cload(atd1_t, [128, HID], bf16)
            b1r_sb = cload(b1r_t, [128, HID], f32)
            b2r_sb = cload(b2r_t, [128, HID], f32)
            idf_sb = cload(idf_t, [128, 128], f32)
            xsh_sb = cload(xsh_t, [128, NBLK], i32)

            adsb1 = cpool.tile([128, NBLK, HEADS], bf16, tag="adsb1")
            adsb2 = cpool.tile([128, NBLK, HEADS], bf16, tag="adsb2")
            adsb = [adsb1, adsb2]

            # ---- phase A: T = emb @ W1.T for own rows (sequential) --------
            for g in range(NBLK):
                ex = wpool.tile([EMB_IN, 128], bf16, tag="embx")
                nc.sync.dma_start(ex[:], embt_t[:, g * 128:(g + 1) * 128])
                hp = ppool.tile([128, HID], f32, tag="hp")
                nc.tensor.matmul(out=hp[:], lhsT=ex[:], rhs=w1t_sb[:],
                                 start=True, stop=True)
                sl = wpool.tile([128, HID], bf16, tag="slice")
                nc.scalar.copy(sl[:], hp[:])
                nc.sync.dma_start(gslice[0][g * 128:(g + 1) * 128, :], sl[:])

            nc.gpsimd.collective_compute(
                "AllGather", mybir.AluOpType.bypass,
                replica_groups=[list(range(NCORES))],
                ins=[gslice[0].ap()], outs=[gfull[0].ap()])

            # ---- a_dst for layer 1: gather T[x[own]], dot att_dst ---------
            for g in range(NBLK):
                gx = wpool.tile([128, HID], bf16, tag="gx")
                nc.gpsimd.indirect_dma_start(
                    out=gx[:], out_offset=None, in_=gfull[0][:],
                    in_offset=IndirectOffsetOnAxis(ap=xsh_sb[:, g:g + 1], axis=0))
                hd = wpool.tile([128, HID], bf16, tag="hd")
                nc.vector.tensor_mul(hd[:], gx[:], atd1_sb[:])
                adf = wpool.tile([128, HEADS], f32, tag="adf")
                nc.vector.tensor_reduce(
                    adf[:], hd[:].rearrange("p (h c) -> p h c", h=HEADS),
                    axis=mybir.AxisListType.X, op=mybir.AluOpType.add)
                nc.scalar.copy(adsb1[:, g, :], adf[:])

            # ---- edge phase -----------------------------------------------
            for layer in (0, 1):
                L = lay[layer]
                gf = gfull[layer]
                gg0 = 0
                for si, blks in enumerate(sups):
                    cw, bg = L["plan"][si]
                    Ws = sum(cw)
                    idxt = wpool.tile([128, Ws * 8], i16, tag="idx")
                    nc.sync.dma_start(
                        idxt[:], gidx_t[layer][:, gg0 * 8:(gg0 + Ws) * 8])
                    gath = wpool.tile([128, Ws, HID], bf16, tag="gath")
                    co = 0
                    for c in range(NCHUNK):
                        Wc = cw[c]
                        if Wc == 0:
                            continue
                        nc.gpsimd.dma_gather(
                            out_ap=gath[:, co:co + Wc, :],
                            in_ap=gf[c * CHUNK:(c + 1) * CHUNK, :],
                            idxs_ap=idxt[:, co * 8:(co + Wc) * 8],
                            num_idxs=Wc * 128,
                            num_idxs_reg=Wc * 128,
                            elem_size=HID,
                            single_packet=False,
                        )
                        co += Wc
                    sdt_sb = wpool.tile([128, Ws, 128], f8, tag="sdt")
                    nc.sync.dma_start(
                        sdt_sb[:],
                        sdt_t[layer][:, gg0 * 128:(gg0 + Ws) * 128]
                        .rearrange("p (w e) -> p w e", w=Ws))
                    s01_sb = wpool.tile([128, Ws, 128], f8, tag="s01")
                    nc.sync.dma_start(
                        s01_sb[:],
                        s01_t[layer][:, gg0 * 128:(gg0 + Ws) * 128]
                        .rearrange("p (w e) -> p w e", w=Ws))
                    # d_e = a_dst[dst_e] via fp8 selection matmul
                    dps = ppool.tile([128, Ws, HEADS], f32, tag="dp")
                    for b in blks:
                        for gc in bg[b]:
                            jj = gc - gg0
                            nc.tensor.matmul(out=dps[:, jj, :],
                                             lhsT=sdt_sb[:, jj, :],
                                             rhs=adsb[layer][:, b, :],
                                             start=True, stop=True)
                    # s_e = <h_src, att_src>  (hm staged in rhs[:, :, 0:HID])
                    rhs = wpool.tile([128, Ws, HID + HEADS], bf16, tag="rhs")
                    nc.vector.tensor_mul(rhs[:, :, 0:HID], gath[:],
                                         _bmid(at_sb[layer][:], Ws))
                    s_sb = wpool.tile([128, Ws, HEADS], f32, tag="s")
                    nc.vector.tensor_reduce(
                        s_sb[:],
                        rhs[:, :, 0:HID].rearrange("p w (h c) -> p w h c",
                                                   h=HEADS),
                        axis=mybir.AxisListType.X, op=mybir.AluOpType.add)
                    # ex = exp(leaky_relu(s+d)) = max(exp(z), exp(NEG*z))
                    z = wpool.tile([128, Ws, HEADS], f32, tag="z")
                    nc.vector.tensor_add(z[:], s_sb[:], dps[:])
                    e1 = wpool.tile([128, Ws, HEADS], bf16, tag="e1")
                    nc.scalar.activation(e1[:], z[:],
                                         mybir.ActivationFunctionType.Exp)
                    e2 = wpool.tile([128, Ws, HEADS], bf16, tag="e2")
                    nc.scalar.activation(e2[:], z[:],
                                         mybir.ActivationFunctionType.Exp,
                                         scale=NEG)
                    nc.vector.tensor_max(rhs[:, :, HID:HID + HEADS],
                                         e1[:], e2[:])
                    # rhs[:, :, 0:HID] = h * ex
                    nc.vector.tensor_mul(
                        rhs[:, :, 0:HID].rearrange("p w (h c) -> p w h c",
                                                   h=HEADS),
                        gath[:].rearrange("p w (h c) -> p w h c", h=HEADS),
                        rhs[:, :, HID:HID + HEADS]
                        .to_broadcast([128, Ws, HEADS, C]))
                    # aggregate per block
                    for b in blks:
                        ggs = bg[b]
                        agg = ppool.tile([128, HID + HEADS], f32, tag="agg")
                        for i, gc in enumerate(ggs):
                            jj = gc - gg0
                            nc.tensor.matmul(out=agg[:], lhsT=s01_sb[:, jj, :],
                                             rhs=rhs[:, jj, :],
                                             start=(i == 0),
                                             stop=(i == len(ggs) - 1))
                        den = wpool.tile([128, HEADS], f32, tag="den")
                        nc.vector.tensor_scalar_add(
                            den[:], agg[:, HID:HID + HEADS], 1e-16)
                        rec = wpool.tile([128, HEADS], f32, tag="rec")
                        nc.vector.reciprocal(rec[:], den[:])
                        normed = wpool.tile([128, HID], f32, tag="normed")
                        nc.vector.tensor_mul(
                            normed[:].rearrange("p (h c) -> p h c", h=HEADS),
                            agg[:, 0:HID].rearrange("p (h c) -> p h c",
                                                    h=HEADS),
                            rec[:].to_broadcast([128, HEADS, C]))
                        if layer == 0:
                            nc.vector.tensor_add(normed[:], normed[:],
                                                 b1r_sb[:])
                            relu = wpool.tile([128, HID], f32, tag="relu")
                            nc.scalar.activation(
                                relu[:], normed[:],
                                mybir.ActivationFunctionType.Relu)
                            tp2 = ppool.tile([128, 128], f32, tag="tp")
                            nc.tensor.transpose(tp2[:], relu[:], idf_sb[:])
                            rT = wpool.tile([128, 128], bf16, tag="rT")
                            nc.scalar.copy(rT[:], tp2[:])
                            hp2 = ppool.tile([128, HID + HEADS], f32,
                                             tag="hp")
                            nc.tensor.matmul(out=hp2[:, 0:HID], lhsT=rT[:],
                                             rhs=w2t_sb[:], start=True,
                                             stop=True)
                            nc.tensor.matmul(out=hp2[:, HID:HID + HEADS],
                                             lhsT=rT[:], rhs=m2_sb[:],
                                             start=True, stop=True)
                            sl2 = wpool.tile([128, HID], bf16, tag="slice")
                            nc.scalar.copy(sl2[:], hp2[:, 0:HID])
                            nc.vector.tensor_copy(adsb2[:, b, :],
                                                  hp2[:, HID:HID + HEADS])
                            nc.sync.dma_start(
                                gslice[1][b * 128:(b + 1) * 128, :], sl2[:])
                        else:
                            outb = wpool.tile([128, HID], f32, tag="outb")
                            nc.vector.tensor_add(outb[:], normed[:],
                                                 b2r_sb[:])
                            nc.sync.dma_start(
                                out_t[b * 128:(b + 1) * 128, :], outb[:])
                    gg0 += Ws
                if layer == 0:
                    nc.gpsimd.collective_compute(
                        "AllGather", mybir.AluOpType.bypass,
                        replica_groups=[list(range(NCORES))],
                        ins=[gslice[1].ap()], outs=[gfull[1].ap()])
    nc.finalize()
    return nc


def kernel(**inputs):
    x = np.asarray(inputs["x"])
    edge_index = np.asarray(inputs["edge_index"])
    emb = np.asarray(inputs["emb"], dtype=np.float32)
    W1 = np.asarray(inputs["W1"], dtype=np.float32)
    W2 = np.asarray(inputs["W2"], dtype=np.float32)
    as1 = np.asarray(inputs["att_src1"], dtype=np.float32)
    ad1 = np.asarray(inputs["att_dst1"], dtype=np.float32)
    as2 = np.asarray(inputs["att_src2"], dtype=np.float32)
    ad2 = np.asarray(inputs["att_dst2"], dtype=np.float32)
    b1 = np.asarray(inputs["b1"], dtype=np.float32)
    b2 = np.asarray(inputs["b2"], dtype=np.float32)

    key = (edge_index.tobytes(), x.tobytes())
    if _cache.get("key") != key:
        meta, percore = _host_layout(x, edge_index)
        nc = _build(bacc.Bacc("TRN2", target_bir_lowering=False, debug=False,
                              enable_asserts=False, num_devices=NCORES), meta)
        _cache.update(key=key, nc=nc, percore=percore)
    nc, percore = _cache["nc"], _cache["percore"]

    embs = []
    for k in range(NCORES):
        sl = np.zeros((EMB_IN, NPAD), dtype=BF16)
        sl[:, :NSH] = emb[k * NSH:(k + 1) * NSH].T.astype(BF16)
        embs.append(np.ascontiguousarray(sl))

    common = dict(
        w1t=np.ascontiguousarray(W1.T).astype(BF16),
        w2t=np.ascontiguousarray(W2.T).astype(BF16),
        m2=(W2.T @ _amat_d(ad2)).astype(BF16),
        ats1=np.ascontiguousarray(
            np.broadcast_to(as1.reshape(-1), (128, HID))).astype(BF16),
        ats2=np.ascontiguousarray(
            np.broadcast_to(as2.reshape(-1), (128, HID))).astype(BF16),
        atd1=np.ascontiguousarray(
            np.broadcast_to(ad1.reshape(-1), (128, HID))).astype(BF16),
        b1r=np.ascontiguousarray(
            np.broadcast_to(b1, (128, HID))).astype(np.float32),
        b2r=np.ascontiguousarray(
            np.broadcast_to(b2, (128, HID))).astype(np.float32),
        identf=np.eye(128, dtype=np.float32),
    )
    in_maps = [dict(common, embt=embs[k], **percore[k]) for k in range(NCORES)]

    res = run_bass_kernel_spmd(nc, in_maps, core_ids=list(range(NCORES)))
    global LAST_EXEC_NS, LAST_RESULT
    if getattr(res, "exec_time_ns", None) is not None:
        LAST_EXEC_NS = res.exec_time_ns
        LAST_RESULT = res
    out = np.concatenate([res.results[k]["out2"][:NSH] for k in range(NCORES)],
                         axis=0)
    return out.astype(np.float32)


def _amat_d(adst):
    A = np.zeros((HID, HEADS), dtype=np.float32)
    for h in range(HEADS):
        A[h * C:(h + 1) * C, h] = adst[h]
    return A


if __name__ == "__main__":
    import reference
    inputs = {k: np.asarray(v) for k, v in reference.setup_inputs().items()}
    got = kernel(**inputs)
    print("out shape", got.shape, got.dtype)
